# revision 2
# baseline (speedup 1.0000x reference)
"""Causal multi-head attention (B=4, H=16, S=2048, D=64) on 8 TRN2 NeuronCores.

Sharding: B*H = 64 (batch, head) pairs -> 8 per core, fully independent,
no collectives.

Per-core algorithm, processing heads in pairs (hA, hB):
  - Cast Q of both heads side by side into one [S, 128] bf16 DRAM scratch
    (hA in cols 0:64, hB in 64:128); same for K. One XBAR DMA-transpose per
    scratch yields QT/KT [128, S] where partitions 0:64 hold hA's [d, s]
    and 64:128 hold hB's. No zero padding, half the transpose traffic.
  - Scores per head use the 64-partition half: for each k-block kb,
    S^T[kb] = matmul(lhsT=KT[P0:P0+64, kb], rhs=QT[P0:P0+64, q >= kb*128])
    into PSUM [128, <=1536], then one exp(0.125 x) activation per chunk
    -> U^T (bf16, unnormalized probs, transposed). Diagonal block masked
    by upper-triangular multiply on the Pool engine.
  - PV per q-block: O[qb] = sum_kb U^T[kb].T @ [V[kb] | 1] accumulated in
    PSUM [128, 65]; col 64 is the softmax denominator. Normalize with
    per-partition reciprocal multiply (DVE), store f32 via sync HWDGE.

Queue discipline: the single SWDGE queue (gpsimd) carries only cast DMAs;
XBAR transposes and output stores ride the sync HWDGE ring, so transpose
waits never chain behind unrelated casts.
"""

import numpy as np

import concourse.bass as bass
import concourse.tile as tile
from concourse import mybir
from concourse.bass_utils import run_bass_kernel_spmd
from concourse.masks import make_upper_triangular
from concourse.vector_clock import ScopedClock, VectorClock

F32 = mybir.dt.float32
BF16 = mybir.dt.bfloat16

B, H, S, D = 4, 16, 2048, 64
N_CORES = 8
HEADS_PER_CORE = B * H // N_CORES  # 8
N_PAIRS = HEADS_PER_CORE // 2  # 4
NB = S // 128  # 16 blocks of 128
SCALE = 1.0 / np.sqrt(np.float32(D))  # 0.125
CHUNK = 1536  # activation chunk (3 PSUM banks)


def _patch_tile_drain():
    """This walrus build rejects >1 sem wait on the kernel-tail Drain
    instruction ("Too many sync wait commands"). Spread the waits across
    single-wait NOPs on the sync engine instead."""
    if getattr(tile.TileContext, "_drain_patched", False):
        return

    def _drain_and_barrier(self, tick_clock, wait_clock):
        gc = tick_clock.global_clock
        n = len(gc)
        for i in range(n):
            if gc[i] > 0:
                vc = VectorClock([gc[j] if j == i else 0 for j in range(n)])
                nop_inst = self.nc.sync.nop(nofuse=True, hint=f"drainwait{i}")
                wait_clock.add_sem_waits(nop_inst.ins, ScopedClock({None: vc}))
        self.nc.sync.drain()
        self.nc.all_engine_barrier()
        popped = self.nc._tile_sem_poison_stack.pop()
        assert popped is self._sem_poison
        self.nc.clear_and_free_semaphores(list(self.sems.allocated().values()))
        self.nc.all_engine_barrier()

    tile.TileContext._drain_and_barrier = _drain_and_barrier
    tile.TileContext._drain_patched = True


_patch_tile_drain()


def _split_multi_waits(nc, limit=1):
    """This walrus build allows at most one sem wait per instruction.
    Move excess waits onto same-engine NOPs inserted just before."""
    ctr = [0]
    for func in nc.m.functions:
        for bb in func.blocks:
            insts = list(bb.instructions)
            out = []
            changed = False
            for inst in insts:
                si = inst.sync_info
                if si is not None and si.on_wait is not None and len(si.on_wait) > limit:
                    waits = list(si.on_wait)
                    extra, keep = waits[:-limit], waits[-limit:]
                    for w in extra:
                        ctr[0] += 1
                        nop = mybir.InstNoOp(
                            name=f"waitsplit-{ctr[0]}", ins=[], outs=[]
                        )
                        nop.engine = inst.engine
                        nop.sync_info = mybir.SyncInfo(on_wait=[w], on_update=[])
                        out.append(nop)
                    inst.sync_info = mybir.SyncInfo(
                        on_wait=keep, on_update=list(si.on_update or [])
                    )
                    changed = True
                out.append(inst)
            if changed:
                try:
                    bb.instructions[:] = out
                except Exception:
                    bb.instructions = out
    return nc


def build_nc(n_heads: int = HEADS_PER_CORE):
    n_pairs = n_heads // 2
    nc = bass.Bass("TRN2", target_bir_lowering=False)
    q_d = nc.dram_tensor("queries", [n_heads, S, D], F32, kind="ExternalInput")
    k_d = nc.dram_tensor("keys", [n_heads, S, D], F32, kind="ExternalInput")
    v_d = nc.dram_tensor("values", [n_heads, S, D], F32, kind="ExternalInput")
    o_d = nc.dram_tensor("out", [n_heads, S, D], F32, kind="ExternalOutput")

    # [h, p, n, d] views: s = n*128 + p
    v_r = v_d[:].rearrange("h (n p) d -> h p n d", p=128)
    o_r = o_d[:].rearrange("h (n p) d -> h p n d", p=128)

    with tile.TileContext(nc) as tc:
        with (
            tc.tile_pool(name="const", bufs=1) as constp,
            tc.tile_pool(name="scr", bufs=2, space="DRAM") as scrp,
            tc.tile_pool(name="tp", bufs=2) as tpp,
            tc.tile_pool(name="vpool", bufs=4) as vpp,
            tc.tile_pool(name="ut", bufs=2) as utp,
            tc.tile_pool(name="oh", bufs=3) as ohp,
            tc.tile_pool(name="rz", bufs=4) as rzp,
            tc.tile_pool(name="ps_s", bufs=2, space="PSUM") as ps_s,
            tc.tile_pool(name="ps_o", bufs=2, space="PSUM") as ps_o,
        ):
            trimask = constp.tile([128, 128], BF16)
            make_upper_triangular(nc, trimask, val=1.0, diag=True)

            scrs = {}
            tps = {}
            vps = {}

            def issue_casts(p):
                hA, hB = 2 * p, 2 * p + 1
                scrq = scrp.tile([S, 128], BF16, tag="scrq")
                scrk = scrp.tile([S, 128], BF16, tag="scrk")
                nc.gpsimd.dma_start(out=scrq[:, 0:D], in_=q_d[hA])
                nc.gpsimd.dma_start(out=scrq[:, D : 2 * D], in_=q_d[hB])
                nc.gpsimd.dma_start(out=scrk[:, 0:D], in_=k_d[hA])
                nc.gpsimd.dma_start(out=scrk[:, D : 2 * D], in_=k_d[hB])
                scrs[p] = (scrq, scrk)

            def issue_v(p):
                for h in (2 * p, 2 * p + 1):
                    vp = vpp.tile([128, NB, D + 1], BF16, tag="vp")
                    nc.gpsimd.dma_start(out=vp[:, :, 0:D], in_=v_r[h])
                    nc.vector.memset(vp[:, :, D : D + 1], 1.0)
                    vps[h] = vp

            def issue_xbar(p):
                scrq, scrk = scrs.pop(p)
                qt = tpp.tile([128, S], BF16, tag="qt")
                kt = tpp.tile([128, S], BF16, tag="kt")
                nc.sync.dma_start(out=qt, in_=scrq[:, :], transpose=True)
                nc.sync.dma_start(out=kt, in_=scrk[:, :], transpose=True)
                tps[p] = (qt, kt)

            PAIR_AHEAD = 2
            for p in range(min(PAIR_AHEAD, n_pairs)):
                issue_casts(p)
                issue_v(p)
            for p in range(min(PAIR_AHEAD, n_pairs)):
                issue_xbar(p)

            for p in range(n_pairs):
                if p + PAIR_AHEAD < n_pairs:
                    issue_casts(p + PAIR_AHEAD)
                    issue_v(p + PAIR_AHEAD)
                    issue_xbar(p + PAIR_AHEAD)
                qt, kt = tps.pop(p)
                for half in (0, 1):
                    h = 2 * p + half
                    P0 = half * 64
                    vp = vps.pop(h)

                    # --- scores + exp, per k-block ---
                    uts = []
                    for kb in range(NB):
                        L = S - kb * 128  # valid q length (q >= kb*128)
                        ut = utp.tile([128, L], BF16, tag=f"ut{kb}")
                        uts.append(ut)
                        off = 0
                        while off < L:
                            tl = min(CHUNK, L - off)
                            ps = ps_s.tile([128, CHUNK], F32, tag="s")
                            for c0 in range(0, tl, 512):
                                cl = min(512, tl - c0)
                                q0 = kb * 128 + off + c0
                                nc.tensor.matmul(
                                    ps[:, c0 : c0 + cl],
                                    lhsT=kt[P0 : P0 + 64, kb * 128 : (kb + 1) * 128],
                                    rhs=qt[P0 : P0 + 64, q0 : q0 + cl],
                                    start=True,
                                    stop=True,
                                )
                            nc.scalar.activation(
                                out=ut[:, off : off + tl],
                                in_=ps[:, 0:tl],
                                func=mybir.ActivationFunctionType.Exp,
                                scale=float(SCALE),
                            )
                            off += tl
                        # mask diagonal block: keep k <= q (partition <= free)
                        nc.gpsimd.tensor_mul(ut[:, 0:128], ut[:, 0:128], trimask)

                    # --- O = P @ [V | 1], per q-block ---
                    oh = ohp.tile([128, NB, D], F32, tag="oh")
                    for qb in range(NB):
                        po = ps_o.tile([128, D + 1], F32, tag="o")
                        for kb in range(qb + 1):
                            nc.tensor.matmul(
                                po,
                                lhsT=uts[kb][:, (qb - kb) * 128 : (qb - kb) * 128 + 128],
                                rhs=vp[:, kb, :],
                                start=(kb == 0),
                                stop=(kb == qb),
                            )
                        rz = rzp.tile([128, 1], F32, tag="rz")
                        nc.vector.reciprocal(rz, po[:, D : D + 1])
                        nc.vector.tensor_scalar_mul(oh[:, qb, :], po[:, 0:D], rz)
                    nc.sync.dma_start(out=o_r[h], in_=oh)
    _split_multi_waits(nc)
    return nc


_NC_CACHE = {}


def _get_nc(n_heads: int = HEADS_PER_CORE):
    if n_heads not in _NC_CACHE:
        _NC_CACHE[n_heads] = build_nc(n_heads)
    return _NC_CACHE[n_heads]


def make_in_maps(queries, keys, values):
    qf = np.ascontiguousarray(
        np.asarray(queries, dtype=np.float32).reshape(B * H, S, D)
    )
    kf = np.ascontiguousarray(np.asarray(keys, dtype=np.float32).reshape(B * H, S, D))
    vf = np.ascontiguousarray(
        np.asarray(values, dtype=np.float32).reshape(B * H, S, D)
    )
    n = HEADS_PER_CORE
    return [
        {
            "queries": qf[i * n : (i + 1) * n],
            "keys": kf[i * n : (i + 1) * n],
            "values": vf[i * n : (i + 1) * n],
        }
        for i in range(N_CORES)
    ]


def kernel(keys, queries, values, head_dim=None, **_ignored):
    nc = _get_nc()
    in_maps = make_in_maps(queries, keys, values)
    res = run_bass_kernel_spmd(nc, in_maps, core_ids=list(range(N_CORES)))
    out = np.concatenate([res.results[i]["out"] for i in range(N_CORES)], axis=0)
    return out.reshape(B, H, S, D).astype(np.float32)


# revision 3
# speedup vs baseline: 1.2318x; 1.2318x over previous
"""Causal multi-head attention (B=4, H=16, S=2048, D=64) on 8 TRN2 NeuronCores.

Sharding: B*H = 64 (batch, head) pairs -> 8 per core, fully independent,
no collectives.

Per-core algorithm, processing heads in pairs (hA, hB):
  - Cast Q of both heads side by side into one [S, 128] bf16 DRAM scratch
    (hA in cols 0:64, hB in 64:128); same for K. One XBAR DMA-transpose per
    scratch yields QT/KT [128, S] where partitions 0:64 hold hA's [d, s]
    and 64:128 hold hB's. No zero padding, half the transpose traffic.
  - Scores per head use the 64-partition half: for each k-block kb,
    S^T[kb] = matmul(lhsT=KT[P0:P0+64, kb], rhs=QT[P0:P0+64, q >= kb*128])
    into PSUM [128, <=1536], then one exp(0.125 x) activation per chunk
    -> U^T (bf16, unnormalized probs, transposed). Diagonal block masked
    by upper-triangular multiply on the Pool engine.
  - PV per q-block: O[qb] = sum_kb U^T[kb].T @ [V[kb] | 1] accumulated in
    PSUM [128, 65]; col 64 is the softmax denominator. Normalize with
    per-partition reciprocal multiply (DVE), store f32 via sync HWDGE.

Queue discipline: the single SWDGE queue (gpsimd) carries only cast DMAs;
XBAR transposes and output stores ride the sync HWDGE ring, so transpose
waits never chain behind unrelated casts.
"""

import numpy as np

import concourse.bass as bass
import concourse.tile as tile
from concourse import mybir
from concourse.bass_utils import run_bass_kernel_spmd
from concourse.masks import make_upper_triangular
from concourse.vector_clock import ScopedClock, VectorClock

F32 = mybir.dt.float32
BF16 = mybir.dt.bfloat16

B, H, S, D = 4, 16, 2048, 64
N_CORES = 8
HEADS_PER_CORE = B * H // N_CORES  # 8
N_PAIRS = HEADS_PER_CORE // 2  # 4
NB = S // 128  # 16 blocks of 128
SCALE = 1.0 / np.sqrt(np.float32(D))  # 0.125
CHUNK = 1024  # activation chunk (2 PSUM banks)


def _patch_tile_drain():
    """This walrus build rejects >1 sem wait on the kernel-tail Drain
    instruction ("Too many sync wait commands"). Spread the waits across
    single-wait NOPs on the sync engine instead."""
    if getattr(tile.TileContext, "_drain_patched", False):
        return

    def _drain_and_barrier(self, tick_clock, wait_clock):
        gc = tick_clock.global_clock
        n = len(gc)
        for i in range(n):
            if gc[i] > 0:
                vc = VectorClock([gc[j] if j == i else 0 for j in range(n)])
                nop_inst = self.nc.sync.nop(nofuse=True, hint=f"drainwait{i}")
                wait_clock.add_sem_waits(nop_inst.ins, ScopedClock({None: vc}))
        self.nc.sync.drain()
        self.nc.all_engine_barrier()
        popped = self.nc._tile_sem_poison_stack.pop()
        assert popped is self._sem_poison
        self.nc.clear_and_free_semaphores(list(self.sems.allocated().values()))
        self.nc.all_engine_barrier()

    tile.TileContext._drain_and_barrier = _drain_and_barrier
    tile.TileContext._drain_patched = True


_patch_tile_drain()


def _split_multi_waits(nc, limit=1):
    """This walrus build allows at most one sem wait per instruction.
    Move excess waits onto same-engine NOPs inserted just before."""
    ctr = [0]
    for func in nc.m.functions:
        for bb in func.blocks:
            insts = list(bb.instructions)
            out = []
            changed = False
            for inst in insts:
                si = inst.sync_info
                if si is not None and si.on_wait is not None and len(si.on_wait) > limit:
                    waits = list(si.on_wait)
                    extra, keep = waits[:-limit], waits[-limit:]
                    for w in extra:
                        ctr[0] += 1
                        nop = mybir.InstNoOp(
                            name=f"waitsplit-{ctr[0]}", ins=[], outs=[]
                        )
                        nop.engine = inst.engine
                        nop.sync_info = mybir.SyncInfo(on_wait=[w], on_update=[])
                        out.append(nop)
                    inst.sync_info = mybir.SyncInfo(
                        on_wait=keep, on_update=list(si.on_update or [])
                    )
                    changed = True
                out.append(inst)
            if changed:
                try:
                    bb.instructions[:] = out
                except Exception:
                    bb.instructions = out
    return nc


def build_nc(n_heads: int = HEADS_PER_CORE):
    n_pairs = n_heads // 2
    nc = bass.Bass("TRN2", target_bir_lowering=False)
    q_d = nc.dram_tensor("queries", [n_heads, S, D], F32, kind="ExternalInput")
    k_d = nc.dram_tensor("keys", [n_heads, S, D], F32, kind="ExternalInput")
    v_d = nc.dram_tensor("values", [n_heads, S, D], F32, kind="ExternalInput")
    o_d = nc.dram_tensor("out", [n_heads, S, D], F32, kind="ExternalOutput")

    # [h, p, n, d] views: s = n*128 + p
    v_r = v_d[:].rearrange("h (n p) d -> h p n d", p=128)
    o_r = o_d[:].rearrange("h (n p) d -> h p n d", p=128)

    with tile.TileContext(nc) as tc:
        with (
            tc.tile_pool(name="const", bufs=1) as constp,
            tc.tile_pool(name="scr", bufs=3, space="DRAM") as scrp,
            tc.tile_pool(name="tp", bufs=3) as tpp,
            tc.tile_pool(name="vpool", bufs=6) as vpp,
            tc.tile_pool(name="ut", bufs=3) as utp,
            tc.tile_pool(name="oh", bufs=3) as ohp,
            tc.tile_pool(name="rz", bufs=4) as rzp,
            tc.tile_pool(name="ps_s", bufs=3, space="PSUM") as ps_s,
            tc.tile_pool(name="ps_o", bufs=2, space="PSUM") as ps_o,
        ):
            trimask = constp.tile([128, 128], BF16)
            make_upper_triangular(nc, trimask, val=1.0, diag=True)

            scrs = {}
            tps = {}
            vps = {}

            def issue_casts_q(p):
                hA, hB = 2 * p, 2 * p + 1
                scrq = scrp.tile([S, 128], BF16, tag="scrq")
                nc.gpsimd.dma_start(out=scrq[:, 0:D], in_=q_d[hA])
                nc.gpsimd.dma_start(out=scrq[:, D : 2 * D], in_=q_d[hB])
                scrs[("q", p)] = scrq

            def issue_casts_k(p):
                hA, hB = 2 * p, 2 * p + 1
                scrk = scrp.tile([S, 128], BF16, tag="scrk")
                nc.gpsimd.dma_start(out=scrk[:, 0:D], in_=k_d[hA])
                nc.gpsimd.dma_start(out=scrk[:, D : 2 * D], in_=k_d[hB])
                scrs[("k", p)] = scrk

            def issue_v(p):
                for h in (2 * p, 2 * p + 1):
                    vp = vpp.tile([128, NB, D + 1], BF16, tag="vp")
                    nc.gpsimd.dma_start(out=vp[:, :, 0:D], in_=v_r[h])
                    nc.vector.memset(vp[:, :, D : D + 1], 1.0)
                    vps[h] = vp

            def issue_xbar_q(p):
                scrq = scrs.pop(("q", p))
                qt = tpp.tile([128, S], BF16, tag="qt")
                nc.sync.dma_start(out=qt, in_=scrq[:, :], transpose=True)
                tps[("q", p)] = qt

            def issue_xbar_k(p):
                scrk = scrs.pop(("k", p))
                kt = tpp.tile([128, S], BF16, tag="kt")
                nc.sync.dma_start(out=kt, in_=scrk[:, :], transpose=True)
                tps[("k", p)] = kt

            def issue_prep(p):
                issue_casts_q(p)
                issue_xbar_q(p)
                issue_casts_k(p)
                issue_xbar_k(p)
                issue_v(p)

            PAIR_AHEAD = 2
            for p in range(min(PAIR_AHEAD, n_pairs)):
                issue_prep(p)

            for p in range(n_pairs):
                if p + PAIR_AHEAD < n_pairs:
                    issue_prep(p + PAIR_AHEAD)
                qt = tps.pop(("q", p))
                kt = tps.pop(("k", p))
                for half in (0, 1):
                    h = 2 * p + half
                    P0 = half * 64
                    vp = vps.pop(h)

                    # --- scores + exp, per k-block ---
                    uts = []
                    for kb in range(NB):
                        L = S - kb * 128  # valid q length (q >= kb*128)
                        ut = utp.tile([128, L], BF16, tag=f"ut{kb}")
                        uts.append(ut)
                        off = 0
                        while off < L:
                            tl = min(CHUNK, L - off)
                            ps = ps_s.tile([128, CHUNK], F32, tag="s")
                            for c0 in range(0, tl, 512):
                                cl = min(512, tl - c0)
                                q0 = kb * 128 + off + c0
                                nc.tensor.matmul(
                                    ps[:, c0 : c0 + cl],
                                    lhsT=kt[P0 : P0 + 64, kb * 128 : (kb + 1) * 128],
                                    rhs=qt[P0 : P0 + 64, q0 : q0 + cl],
                                    start=True,
                                    stop=True,
                                )
                            nc.scalar.activation(
                                out=ut[:, off : off + tl],
                                in_=ps[:, 0:tl],
                                func=mybir.ActivationFunctionType.Exp,
                                scale=float(SCALE),
                            )
                            off += tl
                        # mask diagonal block: keep k <= q (partition <= free)
                        nc.vector.tensor_mul(ut[:, 0:128], ut[:, 0:128], trimask)

                    # --- O = P @ [V | 1], per q-block ---
                    oh = ohp.tile([128, NB, D], F32, tag="oh")
                    for qb in range(NB):
                        po = ps_o.tile([128, D + 1], F32, tag="o")
                        for kb in range(qb + 1):
                            nc.tensor.matmul(
                                po,
                                lhsT=uts[kb][:, (qb - kb) * 128 : (qb - kb) * 128 + 128],
                                rhs=vp[:, kb, :],
                                start=(kb == 0),
                                stop=(kb == qb),
                            )
                        rz = rzp.tile([128, 1], F32, tag="rz")
                        nc.vector.reciprocal(rz, po[:, D : D + 1])
                        nc.vector.tensor_scalar_mul(oh[:, qb, :], po[:, 0:D], rz)
                    nc.sync.dma_start(out=o_r[h], in_=oh)
    _split_multi_waits(nc)
    return nc


_NC_CACHE = {}


def _get_nc(n_heads: int = HEADS_PER_CORE):
    if n_heads not in _NC_CACHE:
        _NC_CACHE[n_heads] = build_nc(n_heads)
    return _NC_CACHE[n_heads]


def make_in_maps(queries, keys, values):
    qf = np.ascontiguousarray(
        np.asarray(queries, dtype=np.float32).reshape(B * H, S, D)
    )
    kf = np.ascontiguousarray(np.asarray(keys, dtype=np.float32).reshape(B * H, S, D))
    vf = np.ascontiguousarray(
        np.asarray(values, dtype=np.float32).reshape(B * H, S, D)
    )
    n = HEADS_PER_CORE
    return [
        {
            "queries": qf[i * n : (i + 1) * n],
            "keys": kf[i * n : (i + 1) * n],
            "values": vf[i * n : (i + 1) * n],
        }
        for i in range(N_CORES)
    ]


def kernel(keys, queries, values, head_dim=None, **_ignored):
    nc = _get_nc()
    in_maps = make_in_maps(queries, keys, values)
    res = run_bass_kernel_spmd(nc, in_maps, core_ids=list(range(N_CORES)))
    out = np.concatenate([res.results[i]["out"] for i in range(N_CORES)], axis=0)
    return out.reshape(B, H, S, D).astype(np.float32)


# revision 4
# speedup vs baseline: 1.2659x; 1.0277x over previous
"""Causal multi-head attention (B=4, H=16, S=2048, D=64) on 8 TRN2 NeuronCores.

Sharding: B*H = 64 (batch, head) pairs -> 8 per core, fully independent,
no collectives.

Per-core algorithm, processing heads in pairs (hA, hB):
  - Q of both heads is cast side by side into one [S, 128] bf16 DRAM
    scratch ([Q_A | Q_B]) and DMA-transposed once -> qt_pair [128, S]
    (partitions 0:64 = Q_A^T, 64:128 = Q_B^T). No zero padding needed:
    the foreign half is annihilated by zero WEIGHT rows.
  - K is cast per head into its own half of a zero-padded [S, 128]
    scratch (K_A in cols 0:64, K_B in cols 64:128; other half zero) and
    transposed -> kt_h [128, S] whose zero rows align with the other
    head's rows of qt_pair. Scores keep the full-rate K=128 contraction:
    matmul(lhsT=kt_h[:, kb], rhs=qt_pair[:, q >= kb*128]).
    Zero halves are only (re)written on the first pass of each scratch
    ring slot - casts never touch them afterwards.
  - exp(0.125 x) on ScalarE per <=1024-col PSUM chunk -> U^T (bf16,
    unnormalized probs, transposed). Diagonal block masked by
    upper-triangular multiply (DVE).
  - PV per q-block: O[qb] = sum_kb U^T[kb].T @ [V[kb] | 1] accumulated
    in PSUM [128, 65]; col 64 is the softmax denominator. Normalize with
    per-partition reciprocal multiply, store f32 via sync HWDGE.

Pipelining: head h's PV matmuls are emitted interleaved into head h+1's
scores loop (PV(h, qb) after scores(h+1, kb=qb)), so ScalarE sees an
uninterrupted stream of score chunks and never idles during a PV tail.
The single SWDGE queue (gpsimd) carries only cast/zpad DMAs; XBAR
transposes and output stores ride the sync HWDGE ring.
"""

import numpy as np

import concourse.bass as bass
import concourse.tile as tile
from concourse import mybir
from concourse.bass_utils import run_bass_kernel_spmd
from concourse.masks import make_upper_triangular
from concourse.vector_clock import ScopedClock, VectorClock

F32 = mybir.dt.float32
BF16 = mybir.dt.bfloat16

B, H, S, D = 4, 16, 2048, 64
N_CORES = 8
HEADS_PER_CORE = B * H // N_CORES  # 8
NB = S // 128  # 16 blocks of 128
SCALE = 1.0 / np.sqrt(np.float32(D))  # 0.125
CHUNK = 1024  # activation chunk (2 PSUM banks)
SCR_BUFS = 3  # scratch ring depth; also how many ring slots get zpadded
PAIR_AHEAD = 2


def _patch_tile_drain():
    """This walrus build rejects >1 sem wait on the kernel-tail Drain
    instruction ("Too many sync wait commands"). Spread the waits across
    single-wait NOPs on the sync engine instead."""
    if getattr(tile.TileContext, "_drain_patched", False):
        return

    def _drain_and_barrier(self, tick_clock, wait_clock):
        gc = tick_clock.global_clock
        n = len(gc)
        for i in range(n):
            if gc[i] > 0:
                vc = VectorClock([gc[j] if j == i else 0 for j in range(n)])
                nop_inst = self.nc.sync.nop(nofuse=True, hint=f"drainwait{i}")
                wait_clock.add_sem_waits(nop_inst.ins, ScopedClock({None: vc}))
        self.nc.sync.drain()
        self.nc.all_engine_barrier()
        popped = self.nc._tile_sem_poison_stack.pop()
        assert popped is self._sem_poison
        self.nc.clear_and_free_semaphores(list(self.sems.allocated().values()))
        self.nc.all_engine_barrier()

    tile.TileContext._drain_and_barrier = _drain_and_barrier
    tile.TileContext._drain_patched = True


_patch_tile_drain()


def _split_multi_waits(nc, limit=1):
    """This walrus build allows at most one sem wait per instruction.
    Move excess waits onto same-engine NOPs inserted just before."""
    ctr = [0]
    for func in nc.m.functions:
        for bb in func.blocks:
            insts = list(bb.instructions)
            out = []
            changed = False
            for inst in insts:
                si = inst.sync_info
                if si is not None and si.on_wait is not None and len(si.on_wait) > limit:
                    waits = list(si.on_wait)
                    extra, keep = waits[:-limit], waits[-limit:]
                    for w in extra:
                        ctr[0] += 1
                        nop = mybir.InstNoOp(
                            name=f"waitsplit-{ctr[0]}", ins=[], outs=[]
                        )
                        nop.engine = inst.engine
                        nop.sync_info = mybir.SyncInfo(on_wait=[w], on_update=[])
                        out.append(nop)
                    inst.sync_info = mybir.SyncInfo(
                        on_wait=keep, on_update=list(si.on_update or [])
                    )
                    changed = True
                out.append(inst)
            if changed:
                try:
                    bb.instructions[:] = out
                except Exception:
                    bb.instructions = out
    return nc


def build_nc(n_heads: int = HEADS_PER_CORE):
    n_pairs = n_heads // 2
    nc = bass.Bass("TRN2", target_bir_lowering=False)
    q_d = nc.dram_tensor("queries", [n_heads, S, D], F32, kind="ExternalInput")
    k_d = nc.dram_tensor("keys", [n_heads, S, D], F32, kind="ExternalInput")
    v_d = nc.dram_tensor("values", [n_heads, S, D], F32, kind="ExternalInput")
    o_d = nc.dram_tensor("out", [n_heads, S, D], F32, kind="ExternalOutput")

    # [h, p, n, d] views: s = n*128 + p
    v_r = v_d[:].rearrange("h (n p) d -> h p n d", p=128)
    o_r = o_d[:].rearrange("h (n p) d -> h p n d", p=128)

    with tile.TileContext(nc) as tc:
        with (
            tc.tile_pool(name="const", bufs=1) as constp,
            tc.tile_pool(name="scr", bufs=SCR_BUFS, space="DRAM") as scrp,
            tc.tile_pool(name="tp", bufs=3) as tpp,
            tc.tile_pool(name="vpool", bufs=6) as vpp,
            tc.tile_pool(name="ut", bufs=3) as utp,
            tc.tile_pool(name="oh", bufs=3) as ohp,
            tc.tile_pool(name="rz", bufs=4) as rzp,
            tc.tile_pool(name="ps_s", bufs=3, space="PSUM") as ps_s,
            tc.tile_pool(name="ps_o", bufs=2, space="PSUM") as ps_o,
        ):
            trimask = constp.tile([128, 128], BF16)
            make_upper_triangular(nc, trimask, val=1.0, diag=True)
            zpad = constp.tile([128, 1024], BF16)
            nc.vector.memset(zpad, 0.0)

            tts = {}
            vps = {}
            nzp = {"ka": 0, "kb": 0}

            def issue_prep(p):
                hA, hB = 2 * p, 2 * p + 1
                # Q pair: [Q_A | Q_B], no padding
                scrq = scrp.tile([S, 128], BF16, tag="scrq")
                nc.gpsimd.dma_start(out=scrq[:, 0:D], in_=q_d[hA])
                nc.gpsimd.dma_start(out=scrq[:, D : 2 * D], in_=q_d[hB])
                qt = tpp.tile([128, S], BF16, tag="qt")
                nc.sync.dma_start(out=qt, in_=scrq[:, :], transpose=True)
                tts[("q", p)] = qt
                # K_A: [K_A | 0]
                scrka = scrp.tile([S, 128], BF16, tag="scrka")
                nc.gpsimd.dma_start(out=scrka[:, 0:D], in_=k_d[hA])
                if nzp["ka"] < SCR_BUFS:
                    nzp["ka"] += 1
                    nc.gpsimd.dma_start(out=scrka[:, D : 2 * D], in_=zpad)
                kta = tpp.tile([128, S], BF16, tag="kta")
                nc.sync.dma_start(out=kta, in_=scrka[:, :], transpose=True)
                tts[("ka", p)] = kta
                # K_B: [0 | K_B]
                scrkb = scrp.tile([S, 128], BF16, tag="scrkb")
                nc.gpsimd.dma_start(out=scrkb[:, D : 2 * D], in_=k_d[hB])
                if nzp["kb"] < SCR_BUFS:
                    nzp["kb"] += 1
                    nc.gpsimd.dma_start(out=scrkb[:, 0:D], in_=zpad)
                ktb = tpp.tile([128, S], BF16, tag="ktb")
                nc.sync.dma_start(out=ktb, in_=scrkb[:, :], transpose=True)
                tts[("kb", p)] = ktb
                # V of both heads
                for h in (hA, hB):
                    vp = vpp.tile([128, NB, D + 1], BF16, tag="vp")
                    nc.gpsimd.dma_start(out=vp[:, :, 0:D], in_=v_r[h])
                    nc.vector.memset(vp[:, :, D : D + 1], 1.0)
                    vps[h] = vp

            def emit_pv_qb(st, qb):
                uts, vp, oh = st["uts"], st["vp"], st["oh"]
                po = ps_o.tile([128, D + 1], F32, tag="o")
                for kb in range(qb + 1):
                    nc.tensor.matmul(
                        po,
                        lhsT=uts[kb][:, (qb - kb) * 128 : (qb - kb) * 128 + 128],
                        rhs=vp[:, kb, :],
                        start=(kb == 0),
                        stop=(kb == qb),
                    )
                rz = rzp.tile([128, 1], F32, tag="rz")
                nc.vector.reciprocal(rz, po[:, D : D + 1])
                nc.vector.tensor_scalar_mul(oh[:, qb, :], po[:, 0:D], rz)

            def finish_pv(st):
                nc.sync.dma_start(out=o_r[st["h"]], in_=st["oh"])

            for p in range(min(PAIR_AHEAD, n_pairs)):
                issue_prep(p)

            prev = None
            for h in range(n_heads):
                p, half = divmod(h, 2)
                if half == 0 and p + PAIR_AHEAD < n_pairs:
                    issue_prep(p + PAIR_AHEAD)
                qt = tts[("q", p)]
                kt = tts[("ka", p)] if half == 0 else tts[("kb", p)]
                vp = vps.pop(h)

                uts = []
                for kb in range(NB):
                    L = S - kb * 128  # valid q length (q >= kb*128)
                    ut = utp.tile([128, L], BF16, tag=f"ut{kb}")
                    uts.append(ut)
                    off = 0
                    while off < L:
                        tl = min(CHUNK, L - off)
                        ps = ps_s.tile([128, CHUNK], F32, tag="s")
                        for c0 in range(0, tl, 512):
                            cl = min(512, tl - c0)
                            q0 = kb * 128 + off + c0
                            nc.tensor.matmul(
                                ps[:, c0 : c0 + cl],
                                lhsT=kt[:, kb * 128 : (kb + 1) * 128],
                                rhs=qt[:, q0 : q0 + cl],
                                start=True,
                                stop=True,
                            )
                        nc.scalar.activation(
                            out=ut[:, off : off + tl],
                            in_=ps[:, 0:tl],
                            func=mybir.ActivationFunctionType.Exp,
                            scale=float(SCALE),
                        )
                        off += tl
                    # mask diagonal block: keep k <= q (partition <= free)
                    nc.vector.tensor_mul(ut[:, 0:128], ut[:, 0:128], trimask)
                    # interleave previous head's PV so ScalarE never idles
                    if prev is not None:
                        emit_pv_qb(prev, kb)

                if prev is not None:
                    finish_pv(prev)
                oh = ohp.tile([128, NB, D], F32, tag="oh")
                prev = {"uts": uts, "vp": vp, "oh": oh, "h": h}

            for qb in range(NB):
                emit_pv_qb(prev, qb)
            finish_pv(prev)
    _split_multi_waits(nc)
    return nc


_NC_CACHE = {}


def _get_nc(n_heads: int = HEADS_PER_CORE):
    if n_heads not in _NC_CACHE:
        _NC_CACHE[n_heads] = build_nc(n_heads)
    return _NC_CACHE[n_heads]


def make_in_maps(queries, keys, values):
    qf = np.ascontiguousarray(
        np.asarray(queries, dtype=np.float32).reshape(B * H, S, D)
    )
    kf = np.ascontiguousarray(np.asarray(keys, dtype=np.float32).reshape(B * H, S, D))
    vf = np.ascontiguousarray(
        np.asarray(values, dtype=np.float32).reshape(B * H, S, D)
    )
    n = HEADS_PER_CORE
    return [
        {
            "queries": qf[i * n : (i + 1) * n],
            "keys": kf[i * n : (i + 1) * n],
            "values": vf[i * n : (i + 1) * n],
        }
        for i in range(N_CORES)
    ]


def kernel(keys, queries, values, head_dim=None, **_ignored):
    nc = _get_nc()
    in_maps = make_in_maps(queries, keys, values)
    res = run_bass_kernel_spmd(nc, in_maps, core_ids=list(range(N_CORES)))
    out = np.concatenate([res.results[i]["out"] for i in range(N_CORES)], axis=0)
    return out.reshape(B, H, S, D).astype(np.float32)


# revision 5
# speedup vs baseline: 1.3303x; 1.0509x over previous
"""Causal multi-head attention (B=4, H=16, S=2048, D=64) on 8 TRN2 NeuronCores.

Sharding: B*H = 64 (batch, head) pairs -> 8 per core, fully independent,
no collectives.

Per-core algorithm, processing heads in pairs (hA, hB):
  - Q of both heads is cast side by side into one [S, 128] bf16 DRAM
    scratch ([Q_A | Q_B]) and DMA-transposed once -> qt_pair [128, S]
    (partitions 0:64 = Q_A^T, 64:128 = Q_B^T). No zero padding needed:
    the foreign half is annihilated by zero WEIGHT rows.
  - K is cast per head into its own half of a zero-padded [S, 128]
    scratch (K_A in cols 0:64, K_B in cols 64:128; other half zero) and
    transposed -> kt_h [128, S] whose zero rows align with the other
    head's rows of qt_pair. Scores keep the full-rate K=128 contraction:
    matmul(lhsT=kt_h[:, kb], rhs=qt_pair[:, q >= kb*128]).
    Zero halves are only (re)written on the first pass of each scratch
    ring slot - casts never touch them afterwards.
  - exp(0.125 x) on ScalarE per <=1024-col PSUM chunk -> U^T (bf16,
    unnormalized probs, transposed). Diagonal block masked by
    upper-triangular multiply (DVE).
  - PV per q-block: O[qb] = sum_kb U^T[kb].T @ [V[kb] | 1] accumulated
    in PSUM [128, 65]; col 64 is the softmax denominator. Normalize with
    per-partition reciprocal multiply, store f32 via sync HWDGE.

Pipelining: head h's PV matmuls are emitted interleaved into head h+1's
scores loop (PV(h, qb) after scores(h+1, kb=qb)), so ScalarE sees an
uninterrupted stream of score chunks and never idles during a PV tail.
The single SWDGE queue (gpsimd) carries only cast/zpad DMAs; XBAR
transposes and output stores ride the sync HWDGE ring.
"""

import numpy as np

import concourse.bass as bass
import concourse.tile as tile
from concourse import mybir
from concourse.bass_utils import run_bass_kernel_spmd
from concourse.masks import make_upper_triangular
from concourse.vector_clock import ScopedClock, VectorClock

F32 = mybir.dt.float32
BF16 = mybir.dt.bfloat16

B, H, S, D = 4, 16, 2048, 64
N_CORES = 8
HEADS_PER_CORE = B * H // N_CORES  # 8
NB = S // 128  # 16 blocks of 128
SCALE = 1.0 / np.sqrt(np.float32(D))  # 0.125
CHUNK = 1024  # activation chunk (2 PSUM banks)
SCR_BUFS = 3  # scratch ring depth; also how many ring slots get zpadded
PAIR_AHEAD = 2


def _patch_tile_drain():
    """This walrus build rejects >1 sem wait on the kernel-tail Drain
    instruction ("Too many sync wait commands"). Spread the waits across
    single-wait NOPs on the sync engine instead."""
    if getattr(tile.TileContext, "_drain_patched", False):
        return

    def _drain_and_barrier(self, tick_clock, wait_clock):
        gc = tick_clock.global_clock
        n = len(gc)
        for i in range(n):
            if gc[i] > 0:
                vc = VectorClock([gc[j] if j == i else 0 for j in range(n)])
                nop_inst = self.nc.sync.nop(nofuse=True, hint=f"drainwait{i}")
                wait_clock.add_sem_waits(nop_inst.ins, ScopedClock({None: vc}))
        self.nc.sync.drain()
        self.nc.all_engine_barrier()
        popped = self.nc._tile_sem_poison_stack.pop()
        assert popped is self._sem_poison
        self.nc.clear_and_free_semaphores(list(self.sems.allocated().values()))
        self.nc.all_engine_barrier()

    tile.TileContext._drain_and_barrier = _drain_and_barrier
    tile.TileContext._drain_patched = True


_patch_tile_drain()


def _split_multi_waits(nc, limit=1):
    """This walrus build allows at most one sem wait per instruction.
    Move excess waits onto same-engine NOPs inserted just before."""
    ctr = [0]
    for func in nc.m.functions:
        for bb in func.blocks:
            insts = list(bb.instructions)
            out = []
            changed = False
            for inst in insts:
                si = inst.sync_info
                if si is not None and si.on_wait is not None and len(si.on_wait) > limit:
                    waits = list(si.on_wait)
                    extra, keep = waits[:-limit], waits[-limit:]
                    for w in extra:
                        ctr[0] += 1
                        nop = mybir.InstNoOp(
                            name=f"waitsplit-{ctr[0]}", ins=[], outs=[]
                        )
                        nop.engine = inst.engine
                        nop.sync_info = mybir.SyncInfo(on_wait=[w], on_update=[])
                        out.append(nop)
                    inst.sync_info = mybir.SyncInfo(
                        on_wait=keep, on_update=list(si.on_update or [])
                    )
                    changed = True
                out.append(inst)
            if changed:
                try:
                    bb.instructions[:] = out
                except Exception:
                    bb.instructions = out
    return nc


def build_nc(n_heads: int = HEADS_PER_CORE):
    n_pairs = n_heads // 2
    nc = bass.Bass("TRN2", target_bir_lowering=False)
    q_d = nc.dram_tensor("queries", [n_heads, S, D], F32, kind="ExternalInput")
    k_d = nc.dram_tensor("keys", [n_heads, S, D], F32, kind="ExternalInput")
    v_d = nc.dram_tensor("values", [n_heads, S, D], F32, kind="ExternalInput")
    o_d = nc.dram_tensor("out", [n_heads, S, D], BF16, kind="ExternalOutput")

    # [h, p, n, d] views: s = n*128 + p
    v_r = v_d[:].rearrange("h (n p) d -> h p n d", p=128)
    o_r = o_d[:].rearrange("h (n p) d -> h p n d", p=128)

    with tile.TileContext(nc) as tc:
        with (
            tc.tile_pool(name="const", bufs=1) as constp,
            tc.tile_pool(name="scr", bufs=SCR_BUFS, space="DRAM") as scrp,
            tc.tile_pool(name="tp", bufs=3) as tpp,
            tc.tile_pool(name="vpool", bufs=8) as vpp,
            tc.tile_pool(name="ut", bufs=3) as utp,
            tc.tile_pool(name="oh", bufs=3) as ohp,
            tc.tile_pool(name="rz", bufs=4) as rzp,
            tc.tile_pool(name="ps_s", bufs=3, space="PSUM") as ps_s,
            tc.tile_pool(name="ps_o", bufs=2, space="PSUM") as ps_o,
        ):
            trimask = constp.tile([128, 128], BF16)
            make_upper_triangular(nc, trimask, val=1.0, diag=True)
            zpad = constp.tile([128, 1024], BF16)
            nc.vector.memset(zpad, 0.0)

            tts = {}
            vps = {}
            nzp = {"ka": 0, "kb": 0}

            def issue_prep_qk(p):
                hA, hB = 2 * p, 2 * p + 1
                # casts feeding head A's first matmuls go first, then the
                # two transposes, then the rest: XBAR transposes and casts
                # serialize on the shared physical DMA queues, so keep the
                # critical chain minimal.
                scrq = scrp.tile([S, 128], BF16, tag="scrq")
                scrka = scrp.tile([S, 128], BF16, tag="scrka")
                nc.gpsimd.dma_start(out=scrq[:, 0:D], in_=q_d[hA])
                nc.gpsimd.dma_start(out=scrq[:, D : 2 * D], in_=q_d[hB])
                nc.gpsimd.dma_start(out=scrka[:, 0:D], in_=k_d[hA])
                if nzp["ka"] < SCR_BUFS:
                    nzp["ka"] += 1
                    nc.gpsimd.dma_start(out=scrka[:, D : 2 * D], in_=zpad)
                qt = tpp.tile([128, S], BF16, tag="qt")
                nc.sync.dma_start(out=qt, in_=scrq[:, :], transpose=True)
                tts[("q", p)] = qt
                kta = tpp.tile([128, S], BF16, tag="kta")
                nc.sync.dma_start(out=kta, in_=scrka[:, :], transpose=True)
                tts[("ka", p)] = kta
                # K_B: [0 | K_B]
                scrkb = scrp.tile([S, 128], BF16, tag="scrkb")
                nc.gpsimd.dma_start(out=scrkb[:, D : 2 * D], in_=k_d[hB])
                if nzp["kb"] < SCR_BUFS:
                    nzp["kb"] += 1
                    nc.gpsimd.dma_start(out=scrkb[:, 0:D], in_=zpad)
                ktb = tpp.tile([128, S], BF16, tag="ktb")
                nc.sync.dma_start(out=ktb, in_=scrkb[:, :], transpose=True)
                tts[("kb", p)] = ktb

            def issue_v(p):
                for h in (2 * p, 2 * p + 1):
                    vp = vpp.tile([128, NB, D + 1], BF16, tag="vp")
                    nc.gpsimd.dma_start(out=vp[:, :, 0:D], in_=v_r[h])
                    nc.vector.memset(vp[:, :, D : D + 1], 1.0)
                    vps[h] = vp

            def emit_pv_qb(st, qb):
                uts, vp, oh = st["uts"], st["vp"], st["oh"]
                po = ps_o.tile([128, D + 1], F32, tag="o")
                for kb in range(qb + 1):
                    nc.tensor.matmul(
                        po,
                        lhsT=uts[kb][:, (qb - kb) * 128 : (qb - kb) * 128 + 128],
                        rhs=vp[:, kb, :],
                        start=(kb == 0),
                        stop=(kb == qb),
                    )
                rz = rzp.tile([128, 1], F32, tag="rz")
                nc.vector.reciprocal(rz, po[:, D : D + 1])
                nc.vector.tensor_scalar_mul(oh[:, qb, :], po[:, 0:D], rz)

            def finish_pv(st):
                nc.sync.dma_start(out=o_r[st["h"]], in_=st["oh"])

            for p in range(min(PAIR_AHEAD, n_pairs)):
                issue_prep_qk(p)
                issue_v(p)

            prev = None
            for h in range(n_heads):
                p, half = divmod(h, 2)
                if half == 0 and p + PAIR_AHEAD < n_pairs:
                    issue_prep_qk(p + PAIR_AHEAD)
                if half == 1 and p + PAIR_AHEAD < n_pairs:
                    issue_v(p + PAIR_AHEAD)
                qt = tts[("q", p)]
                kt = tts[("ka", p)] if half == 0 else tts[("kb", p)]
                vp = vps.pop(h)

                uts = []
                for kb in range(NB):
                    L = S - kb * 128  # valid q length (q >= kb*128)
                    ut = utp.tile([128, L], BF16, tag=f"ut{kb}")
                    uts.append(ut)
                    off = 0
                    while off < L:
                        tl = min(CHUNK, L - off)
                        ps = ps_s.tile([128, CHUNK], F32, tag="s")
                        for c0 in range(0, tl, 512):
                            cl = min(512, tl - c0)
                            q0 = kb * 128 + off + c0
                            nc.tensor.matmul(
                                ps[:, c0 : c0 + cl],
                                lhsT=kt[:, kb * 128 : (kb + 1) * 128],
                                rhs=qt[:, q0 : q0 + cl],
                                start=True,
                                stop=True,
                            )
                        nc.scalar.activation(
                            out=ut[:, off : off + tl],
                            in_=ps[:, 0:tl],
                            func=mybir.ActivationFunctionType.Exp,
                            scale=float(SCALE),
                        )
                        off += tl
                    # mask diagonal block: keep k <= q (partition <= free)
                    nc.vector.tensor_mul(ut[:, 0:128], ut[:, 0:128], trimask)
                    # interleave previous head's PV so ScalarE never idles
                    if prev is not None:
                        emit_pv_qb(prev, kb)

                if prev is not None:
                    finish_pv(prev)
                oh = ohp.tile([128, NB, D], BF16, tag="oh")
                prev = {"uts": uts, "vp": vp, "oh": oh, "h": h}

            for qb in range(NB):
                emit_pv_qb(prev, qb)
            finish_pv(prev)
    _split_multi_waits(nc)
    return nc


_NC_CACHE = {}


def _get_nc(n_heads: int = HEADS_PER_CORE):
    if n_heads not in _NC_CACHE:
        _NC_CACHE[n_heads] = build_nc(n_heads)
    return _NC_CACHE[n_heads]


def make_in_maps(queries, keys, values):
    qf = np.ascontiguousarray(
        np.asarray(queries, dtype=np.float32).reshape(B * H, S, D)
    )
    kf = np.ascontiguousarray(np.asarray(keys, dtype=np.float32).reshape(B * H, S, D))
    vf = np.ascontiguousarray(
        np.asarray(values, dtype=np.float32).reshape(B * H, S, D)
    )
    n = HEADS_PER_CORE
    return [
        {
            "queries": qf[i * n : (i + 1) * n],
            "keys": kf[i * n : (i + 1) * n],
            "values": vf[i * n : (i + 1) * n],
        }
        for i in range(N_CORES)
    ]


def kernel(keys, queries, values, head_dim=None, **_ignored):
    nc = _get_nc()
    in_maps = make_in_maps(queries, keys, values)
    res = run_bass_kernel_spmd(nc, in_maps, core_ids=list(range(N_CORES)))
    out = np.concatenate([res.results[i]["out"] for i in range(N_CORES)], axis=0)
    return out.reshape(B, H, S, D).astype(np.float32)


# revision 8
# speedup vs baseline: 1.5928x; 1.1973x over previous
"""Causal multi-head attention (B=4, H=16, S=2048, D=64) on 8 TRN2 NeuronCores.

Sharding: B*H = 64 (batch, head) pairs -> 8 per core, fully independent,
no collectives.

Per-core algorithm (per head):
  - Q, K, V loaded with f32->bf16 cast DMAs (single SWDGE queue) into
    natural [128, 16, 64] block layout.
  - Q and K are transposed on the PE (16 identity-matmul block transposes
    per tensor into a [64, 1024] bf16 PSUM staging tile, batch-copied to
    SBUF by DVE; GpSimd cannot read PSUM) -> qt/kt [128, S] with [d, s] in rows 0:64. kt rows
    64:128 are zeroed once per ring slot (memset survives ring reuse);
    qt's bottom rows are zeroed once per ring slot too (junk would be
    harmless against zero weights only if finite; fresh SBUF may hold
    NaN bit patterns). No DRAM scratch, no XBAR transpose DMAs: the sync HWDGE
    ring carries only output stores, so nothing chains behind slow
    transpose storms on the shared physical DMA queues.
  - Scores per k-block kb: S^T[kb] = matmul(lhsT=kt[:, kb] (K=128, full
    rate), rhs=qt[:, q >= kb*128]) -> PSUM [128, <=1024], one
    exp(0.125 x) ScalarE activation per chunk -> U^T (bf16, unnormalized
    probs, transposed). Diagonal block masked by upper-triangular
    multiply (DVE).
  - PV per q-block: O[qb] = sum_kb U^T[kb].T @ [V[kb] | 1] accumulated in
    PSUM [128, 65]; col 64 is the softmax denominator. Normalize with
    per-partition reciprocal multiply, store bf16 via sync HWDGE
    (numpy converts to f32).

Pipelining: phase h interleaves, at k-block granularity: scores+exp of
head h, PE block-transposes for head h+1, and PV matmuls of head h-1 -
so ScalarE sees an uninterrupted stream of score chunks while the PE
fills its slack with transposes and PV.
"""

import numpy as np

import concourse.bass as bass
import concourse.tile as tile
from concourse import mybir
from concourse.bass_utils import run_bass_kernel_spmd
from concourse.masks import make_identity, make_upper_triangular
from concourse.vector_clock import ScopedClock, VectorClock

F32 = mybir.dt.float32
BF16 = mybir.dt.bfloat16

B, H, S, D = 4, 16, 2048, 64
N_CORES = 8
HEADS_PER_CORE = B * H // N_CORES  # 8
NB = S // 128  # 16 blocks of 128
SCALE = 1.0 / np.sqrt(np.float32(D))  # 0.125
CHUNK = 1024  # activation chunk (2 PSUM banks)


def _patch_tile_drain():
    """This walrus build rejects >1 sem wait on the kernel-tail Drain
    instruction ("Too many sync wait commands"). Spread the waits across
    single-wait NOPs on the sync engine instead."""
    if getattr(tile.TileContext, "_drain_patched", False):
        return

    def _drain_and_barrier(self, tick_clock, wait_clock):
        gc = tick_clock.global_clock
        n = len(gc)
        for i in range(n):
            if gc[i] > 0:
                vc = VectorClock([gc[j] if j == i else 0 for j in range(n)])
                nop_inst = self.nc.sync.nop(nofuse=True, hint=f"drainwait{i}")
                wait_clock.add_sem_waits(nop_inst.ins, ScopedClock({None: vc}))
        self.nc.sync.drain()
        self.nc.all_engine_barrier()
        popped = self.nc._tile_sem_poison_stack.pop()
        assert popped is self._sem_poison
        self.nc.clear_and_free_semaphores(list(self.sems.allocated().values()))
        self.nc.all_engine_barrier()

    tile.TileContext._drain_and_barrier = _drain_and_barrier
    tile.TileContext._drain_patched = True


_patch_tile_drain()


def _split_multi_waits(nc, limit=1):
    """This walrus build allows at most one sem wait per instruction.
    Move excess waits onto same-engine NOPs inserted just before."""
    ctr = [0]
    for func in nc.m.functions:
        for bb in func.blocks:
            insts = list(bb.instructions)
            out = []
            changed = False
            for inst in insts:
                si = inst.sync_info
                if si is not None and si.on_wait is not None and len(si.on_wait) > limit:
                    waits = list(si.on_wait)
                    extra, keep = waits[:-limit], waits[-limit:]
                    for w in extra:
                        ctr[0] += 1
                        nop = mybir.InstNoOp(
                            name=f"waitsplit-{ctr[0]}", ins=[], outs=[]
                        )
                        nop.engine = inst.engine
                        nop.sync_info = mybir.SyncInfo(on_wait=[w], on_update=[])
                        out.append(nop)
                    inst.sync_info = mybir.SyncInfo(
                        on_wait=keep, on_update=list(si.on_update or [])
                    )
                    changed = True
                out.append(inst)
            if changed:
                try:
                    bb.instructions[:] = out
                except Exception:
                    bb.instructions = out
    return nc


def build_nc(n_heads: int = HEADS_PER_CORE):
    nc = bass.Bass("TRN2", target_bir_lowering=False)
    q_d = nc.dram_tensor("queries", [n_heads, S, D], F32, kind="ExternalInput")
    k_d = nc.dram_tensor("keys", [n_heads, S, D], F32, kind="ExternalInput")
    v_d = nc.dram_tensor("values", [n_heads, S, D], F32, kind="ExternalInput")
    o_d = nc.dram_tensor("out", [n_heads, S, D], BF16, kind="ExternalOutput")

    # [h, p, n, d] views: s = n*128 + p
    q_r = q_d[:].rearrange("h (n p) d -> h p n d", p=128)
    k_r = k_d[:].rearrange("h (n p) d -> h p n d", p=128)
    v_r = v_d[:].rearrange("h (n p) d -> h p n d", p=128)
    o_r = o_d[:].rearrange("h (n p) d -> h p n d", p=128)

    KT_BUFS = 3

    with tile.TileContext(nc) as tc:
        with (
            tc.tile_pool(name="const", bufs=1) as constp,
            tc.tile_pool(name="nat", bufs=3) as natp,
            tc.tile_pool(name="tp", bufs=KT_BUFS) as tpp,
            tc.tile_pool(name="vpool", bufs=4) as vpp,
            tc.tile_pool(name="ut", bufs=3) as utp,
            tc.tile_pool(name="oh", bufs=3) as ohp,
            tc.tile_pool(name="rz", bufs=4) as rzp,
            tc.tile_pool(name="ps_s", bufs=2, space="PSUM") as ps_s,
            tc.tile_pool(name="ps_o", bufs=2, space="PSUM") as ps_o,
            tc.tile_pool(name="ps_t", bufs=2, space="PSUM") as ps_t,
        ):
            trimask = constp.tile([128, 128], BF16)
            make_upper_triangular(nc, trimask, val=1.0, diag=True)
            ident = constp.tile([128, 128], BF16)
            make_identity(nc, ident)

            nats = {}
            tts = {}
            vps = {}
            kt_zeroed = [0]

            def issue_casts(h):
                qn = natp.tile([128, NB, D], BF16, tag="qn")
                nc.gpsimd.dma_start(out=qn, in_=q_r[h])
                kn = natp.tile([128, NB, D], BF16, tag="kn")
                nc.gpsimd.dma_start(out=kn, in_=k_r[h])
                nats[h] = (qn, kn)
                vp = vpp.tile([128, NB, D + 1], BF16, tag="vp")
                nc.gpsimd.dma_start(out=vp[:, :, 0:D], in_=v_r[h])
                nc.vector.memset(vp[:, :, D : D + 1], 1.0)
                vps[h] = vp

            def alloc_tt(h):
                qt = tpp.tile([128, S], BF16, tag="qt")
                kt = tpp.tile([128, S], BF16, tag="kt")
                if kt_zeroed[0] < KT_BUFS:
                    kt_zeroed[0] += 1
                    nc.vector.memset(kt[64:128, :], 0.0)
                    # uninitialized SBUF can hold NaN bit patterns, and
                    # NaN * 0-weight is still NaN - zero qt's junk rows once
                    nc.vector.memset(qt[64:128, :], 0.0)
                tts[h] = (qt, kt)

            def emit_transpose_fill(h, fi):
                """fi 0/1: Q blocks 0-7 / 8-15; fi 2/3: same for K."""
                qn, kn = nats[h]
                qt, kt = tts[h]
                src, dst = (qn, qt) if fi < 2 else (kn, kt)
                base = (fi % 2) * 8
                pt = ps_t.tile([64, 8 * 128], BF16, tag="pt")
                for j in range(8):
                    nc.tensor.transpose(
                        pt[0:64, j * 128 : (j + 1) * 128],
                        src[:, base + j, :],
                        ident,
                    )
                nc.vector.tensor_copy(
                    out=dst[0:64, base * 128 : (base + 8) * 128], in_=pt
                )
                if fi == 3:
                    nats.pop(h)

            def emit_pv_qb(st, qb):
                uts, vp, oh = st["uts"], st["vp"], st["oh"]
                po = ps_o.tile([128, D + 1], F32, tag="o")
                for kb in range(qb + 1):
                    nc.tensor.matmul(
                        po,
                        lhsT=uts[kb][:, (qb - kb) * 128 : (qb - kb) * 128 + 128],
                        rhs=vp[:, kb, :],
                        start=(kb == 0),
                        stop=(kb == qb),
                    )
                rz = rzp.tile([128, 1], F32, tag="rz")
                nc.vector.reciprocal(rz, po[:, D : D + 1])
                nc.vector.tensor_scalar_mul(oh[:, qb, :], po[:, 0:D], rz)

            # startup: casts for heads 0-1, then head 0's transposes upfront
            issue_casts(0)
            issue_casts(1)
            alloc_tt(0)
            for fi in range(4):
                emit_transpose_fill(0, fi)

            prev = None
            for h in range(n_heads):
                if h + 2 < n_heads:
                    issue_casts(h + 2)
                if h + 1 < n_heads:
                    alloc_tt(h + 1)
                qt, kt = tts[h]
                vp = vps.pop(h)

                uts = []
                for kb in range(NB):
                    L = S - kb * 128  # valid q length (q >= kb*128)
                    ut = utp.tile([128, L], BF16, tag=f"ut{kb}")
                    uts.append(ut)
                    off = 0
                    while off < L:
                        tl = min(CHUNK, L - off)
                        ps = ps_s.tile([128, CHUNK], F32, tag="s")
                        for c0 in range(0, tl, 512):
                            cl = min(512, tl - c0)
                            q0 = kb * 128 + off + c0
                            nc.tensor.matmul(
                                ps[:, c0 : c0 + cl],
                                lhsT=kt[:, kb * 128 : (kb + 1) * 128],
                                rhs=qt[:, q0 : q0 + cl],
                                start=True,
                                stop=True,
                            )
                        nc.scalar.activation(
                            out=ut[:, off : off + tl],
                            in_=ps[:, 0:tl],
                            func=mybir.ActivationFunctionType.Exp,
                            scale=float(SCALE),
                        )
                        off += tl
                    # mask diagonal block: keep k <= q (partition <= free)
                    nc.vector.tensor_mul(ut[:, 0:128], ut[:, 0:128], trimask)
                    # next head's PE block-transposes, late in the phase so
                    # its casts have certainly landed
                    if kb in (8, 10, 12, 14) and h + 1 < n_heads:
                        emit_transpose_fill(h + 1, (kb - 8) // 2)
                    # previous head's PV so ScalarE never idles
                    if prev is not None:
                        emit_pv_qb(prev, kb)

                if prev is not None:
                    nc.sync.dma_start(out=o_r[prev["h"]], in_=prev["oh"])
                oh = ohp.tile([128, NB, D], BF16, tag="oh")
                prev = {"uts": uts, "vp": vp, "oh": oh, "h": h}

            for qb in range(NB):
                emit_pv_qb(prev, qb)
            nc.sync.dma_start(out=o_r[prev["h"]], in_=prev["oh"])
    _split_multi_waits(nc)
    return nc


_NC_CACHE = {}


def _get_nc(n_heads: int = HEADS_PER_CORE):
    if n_heads not in _NC_CACHE:
        _NC_CACHE[n_heads] = build_nc(n_heads)
    return _NC_CACHE[n_heads]


def make_in_maps(queries, keys, values):
    qf = np.ascontiguousarray(
        np.asarray(queries, dtype=np.float32).reshape(B * H, S, D)
    )
    kf = np.ascontiguousarray(np.asarray(keys, dtype=np.float32).reshape(B * H, S, D))
    vf = np.ascontiguousarray(
        np.asarray(values, dtype=np.float32).reshape(B * H, S, D)
    )
    n = HEADS_PER_CORE
    return [
        {
            "queries": qf[i * n : (i + 1) * n],
            "keys": kf[i * n : (i + 1) * n],
            "values": vf[i * n : (i + 1) * n],
        }
        for i in range(N_CORES)
    ]


def kernel(keys, queries, values, head_dim=None, **_ignored):
    nc = _get_nc()
    in_maps = make_in_maps(queries, keys, values)
    res = run_bass_kernel_spmd(nc, in_maps, core_ids=list(range(N_CORES)))
    out = np.concatenate([res.results[i]["out"] for i in range(N_CORES)], axis=0)
    return out.reshape(B, H, S, D).astype(np.float32)
